# revision 27
# baseline (speedup 1.0000x reference)
"""Trainium2 Bass kernel for: softmax((hidden@w1+b1) @ ((hidden+pre_emb)@w2+b2)^T).

Shapes: hidden/pre_emb [4, 4096, 1024], w1/w2 [1024,1024], b1/b2 [1024].
Output: [4, 4096, 4096] float32.

Sharding: 8 cores = 4 batches x 2 query-halves. Each core computes
out[b, qh*2048:(qh+1)*2048, :] = softmax over all 4096 keys.

Per-core algorithm (all matmuls in float32r = full-rate ~12-bit-mantissa fp32):
  Keys are processed in 2 rounds of 2048 (SBUF cannot hold i^T for all 4096
  keys plus weights). Each round:
    - build iT_half[e_chunk][128, 2048] = ((hidden+pre_emb) @ w2 + b2)^T via
      PE transposes + 8x8 blocked matmuls
    - per 512-query chunk: transpose hidden_q, proj1 (w1, +b1) -> hT strips
    - per 128-query block: scores psum = hT^T @ iT_half (8 accumulating
      matmuls per 512-key block), fused PSUM->SBUF copy + row-max (DVE
      tensor_tensor_reduce), exp with -max bias + row-sum accum (ACT).
  Round 1 writes unnormalized exp(x - m1) to a DRAM scratch; round 2 combines
  stats (flash-softmax), writes normalized cols 2048:4096, then rescales the
  round-1 half from scratch into cols 0:2048.
"""

import numpy as np

import concourse.bass as bass
import concourse.tile as tile
from concourse import bacc, masks, mybir
from concourse.bass_utils import run_bass_kernel_spmd

F32 = mybir.dt.float32
BF16 = mybir.dt.bfloat16
F32R = mybir.dt.float32r
AF = mybir.ActivationFunctionType
ALU = mybir.AluOpType

B, S, D = 4, 4096, 1024
QP = S // 2          # queries per core = 2048
N_CORES = 8
KH = S // 2          # keys per round = 2048
NEG_INF = -3.0e38

_cache = {}
TRACE = False
LAST_EXEC_NS = None


def _build():
    if "nc" in _cache:
        return _cache["nc"]

    nc = bacc.Bacc("TRN2", target_bir_lowering=False, debug=False,
                   enable_asserts=False, num_devices=N_CORES)

    hid_q = nc.dram_tensor("hid_q", [QP, D], F32, kind="ExternalInput").ap()
    hid_kv = nc.dram_tensor("hid_kv", [S, D], F32, kind="ExternalInput").ap()
    pre_kv = nc.dram_tensor("pre_kv", [S, D], F32, kind="ExternalInput").ap()
    w1_d = nc.dram_tensor("w1", [D, D], F32, kind="ExternalInput").ap()
    w2_d = nc.dram_tensor("w2", [D, D], F32, kind="ExternalInput").ap()
    b1_d = nc.dram_tensor("b1", [D], F32, kind="ExternalInput").ap()
    b2_d = nc.dram_tensor("b2", [D], F32, kind="ExternalInput").ap()
    out_d = nc.dram_tensor("out", [QP, S], F32, kind="ExternalOutput").ap()

    from contextlib import ExitStack
    with tile.TileContext(nc) as tc:
        w1ctx = ExitStack()
        fbctx = ExitStack()
        with tc.tile_pool(name="consts", bufs=1) as consts, \
             tc.tile_pool(name="weights", bufs=1) as weights, \
             tc.tile_pool(name="loads", bufs=2) as loads, \
             tc.tile_pool(name="it", bufs=1) as itpool, \
             tc.tile_pool(name="strips", bufs=1) as strips, \
             tc.tile_pool(name="ht", bufs=2) as htpool, \
             tc.tile_pool(name="sc", bufs=3) as scpool, \
             tc.tile_pool(name="scb", bufs=1) as scbpool, \
             tc.tile_pool(name="keep", bufs=1) as keep, \
             tc.tile_pool(name="st", bufs=1) as stpool, \
             tc.tile_pool(name="dram", bufs=1, space="DRAM") as dpool, \
             tc.tile_pool(name="pstr", bufs=2, space="PSUM") as pstr, \
             tc.tile_pool(name="ppr", bufs=2, space="PSUM") as ppr, \
             tc.tile_pool(name="psc", bufs=4, space="PSUM") as psc:

            ident = consts.tile([128, 128], F32)
            masks.make_identity(nc, ident[:])
            # biases laid out [128, 8]: column mo = b[mo*128:(mo+1)*128]
            b1t = consts.tile([128, 8], F32)
            nc.sync.dma_start(b1t[:], b1_d.rearrange("(a b) -> b a", a=8))
            b2t = consts.tile([128, 8], F32)
            nc.sync.dma_start(b2t[:], b2_d.rearrange("(a b) -> b a", a=8))

            # weights, cast to fp32r: w[ki] = [128 d_in, 1024 d_out]
            # w1 lives in its own pool, freed after round-0 queries
            w1pool = w1ctx.enter_context(tc.tile_pool(name="w1pool", bufs=1))

            def load_weight(wd, pool, wn):
                wr_list = []
                for ki in range(8):
                    wt = loads.tile([128, 2 * D], F32, tag="load", name=f"wt{wn}_{ki}")
                    nc.gpsimd.dma_start(wt[:, 0:D], wd[ki * 128:(ki + 1) * 128, :])
                    wr = pool.tile([128, D], F32R, tag=f"w{wn}_{ki}",
                                   name=f"wr{wn}_{ki}")
                    nc.vector.tensor_copy(wr[:], wt[:, 0:D])
                    wr_list.append(wr)
                return wr_list

            # peel round-0/kt-0 key loads ahead of the weight transfers so the
            # PE's first transposes aren't queued behind 4MB of weights
            pre_lt = []
            for half in range(2):
                r0 = half * 256
                lt = loads.tile([128, 2 * D], F32, tag="load", name=f"pre_lt{half}")
                nc.sync.dma_start(
                    lt[:], hid_kv[r0:r0 + 256, :].rearrange("(j p) c -> p j c", p=128))
                nc.gpsimd.dma_start(
                    lt[:], pre_kv[r0:r0 + 256, :].rearrange("(j p) c -> p j c", p=128),
                    accum_op=ALU.add)
                pre_lt.append(lt)

            w2r = load_weight(w2_d, weights, 2)
            w1r = None  # loaded lazily before the round-0 query phase

            scratch = dpool.tile([QP, KH], BF16)
            # round-0 hT spill: rows mo*128..mo*128+127, cols = query index
            scratch_ht = dpool.tile([8 * 128, QP], F32)

            # per-(qc,qb) saved stats from round 0: cols [2*qi]=rowmax, [2*qi+1]=rowsum
            svt = keep.tile([128, 32], F32, name="svt", tag="svt")
            saved = [svt[:, 2 * i:2 * i + 2] for i in range(16)]

            def transpose_128(src_ap, dst_ap, eng):
                tp = pstr.tile([128, 128], F32, tag="tr")
                nc.tensor.transpose(tp[:], src_ap, ident[:])
                eng(dst_ap, tp[:])

            act_copy = nc.scalar.copy
            vec_copy = nc.vector.tensor_copy

            fbpool = None
            for rnd in range(2):
                k0 = rnd * KH
                if rnd == 1:
                    w1ctx.close()
                    fbpool = fbctx.enter_context(
                        tc.tile_pool(name="fb", bufs=2))
                # ---- build iT for this half of the keys ----
                iT = [itpool.tile([128, KH], F32R, tag=f"it{mo}", name=f"it{rnd}_{mo}")
                      for mo in range(8)]
                for kt in range(4):           # 512-key strips
                    sumT = [strips.tile([128, 512], F32R, tag=f"str{ki}",
                                        name=f"sumT{rnd}_{kt}_{ki}")
                            for ki in range(8)]
                    for half in range(2):     # 256-key load chunks
                        r0 = k0 + kt * 512 + half * 256
                        if rnd == 0 and kt == 0:
                            lt = pre_lt[half]
                        else:
                            lt = loads.tile([128, 2 * D], F32, tag="load",
                                            name=f"lt{rnd}_{kt}_{half}")
                            nc.sync.dma_start(
                                lt[:], hid_kv[r0:r0 + 256, :].rearrange(
                                    "(j p) c -> p j c", p=128))
                            nc.gpsimd.dma_start(
                                lt[:], pre_kv[r0:r0 + 256, :].rearrange(
                                    "(j p) c -> p j c", p=128),
                                accum_op=ALU.add)
                        for j in range(2):
                            st = half * 2 + j
                            for ki in range(8):
                                transpose_128(
                                    lt[:, j * D + ki * 128:j * D + (ki + 1) * 128],
                                    sumT[ki][:, st * 128:(st + 1) * 128],
                                    act_copy if ki % 2 == 0 else vec_copy)
                    for mo in range(8):
                        ps = ppr.tile([128, 512], F32, tag="pr")
                        for ki in range(8):
                            nc.tensor.matmul(ps[:], w2r[ki][:, mo * 128:(mo + 1) * 128],
                                             sumT[ki][:], start=(ki == 0), stop=(ki == 7))
                        nc.scalar.activation(iT[mo][:, kt * 512:(kt + 1) * 512], ps[:],
                                             AF.Identity, bias=b2t[:, mo:mo + 1])

                # ---- queries ----
                if rnd == 0 and w1r is None:
                    w1r = load_weight(w1_d, w1pool, 1)
                for qc in range(4):           # 512-query chunks
                    hT = [htpool.tile([128, 512], F32R, tag=f"ht{mo}",
                                      name=f"hT{rnd}_{qc}_{mo}")
                          for mo in range(8)]
                    if rnd == 0:
                        hqT = [strips.tile([128, 512], F32R, tag=f"str{ki}",
                                           name=f"hqT{rnd}_{qc}_{ki}")
                               for ki in range(8)]
                        for half in range(2):
                            r0 = qc * 512 + half * 256
                            hq = loads.tile([128, 2 * D], F32, tag="load",
                                            name=f"hq{qc}_{half}")
                            nc.sync.dma_start(
                                hq[:], hid_q[r0:r0 + 256, :].rearrange(
                                    "(j p) c -> p j c", p=128))
                            for j in range(2):
                                st = half * 2 + j
                                for ki in range(8):
                                    transpose_128(
                                        hq[:, j * D + ki * 128:j * D + (ki + 1) * 128],
                                        hqT[ki][:, st * 128:(st + 1) * 128],
                                        act_copy if ki % 2 == 0 else vec_copy)
                        for mo in range(8):
                            ps = ppr.tile([128, 512], F32, tag="pr")
                            for ki in range(8):
                                nc.tensor.matmul(ps[:],
                                                 w1r[ki][:, mo * 128:(mo + 1) * 128],
                                                 hqT[ki][:], start=(ki == 0),
                                                 stop=(ki == 7))
                            nc.scalar.activation(hT[mo][:], ps[:], AF.Identity,
                                                 bias=b1t[:, mo:mo + 1])
                            nc.sync.dma_start(
                                scratch_ht[mo * 128:(mo + 1) * 128,
                                           qc * 512:(qc + 1) * 512],
                                hT[mo][:].bitcast(F32))
                    else:
                        for mo in range(8):
                            hl = strips.tile([128, 512], F32, tag=f"str{mo}",
                                             name=f"hl{qc}_{mo}")
                            nc.sync.dma_start(
                                hl[:],
                                scratch_ht[mo * 128:(mo + 1) * 128,
                                           qc * 512:(qc + 1) * 512])
                            nc.vector.tensor_copy(hT[mo][:], hl[:])

                    for qb in range(4):       # 128-query blocks
                        qi = qc * 4 + qb
                        q0 = qc * 512 + qb * 128
                        blockmax = stpool.tile([128, 2], F32, tag="bm")
                        sums = stpool.tile([128, 2], F32, tag="sm")
                        blocks = []
                        for kh in range(2):   # 1024-key blocks
                            blk = scpool.tile([128, 1024], F32, tag="blk",
                                              name=f"blk{rnd}_{qi}_{kh}")
                            for sub in range(2):
                                kb = kh * 2 + sub
                                ps = psc.tile([128, 512], F32, tag="ps")
                                for mo in range(8):
                                    nc.tensor.matmul(
                                        ps[:], hT[mo][:, qb * 128:(qb + 1) * 128],
                                        iT[mo][:, kb * 512:(kb + 1) * 512],
                                        start=(mo == 0), stop=(mo == 7))
                                nc.scalar.copy(blk[:, sub * 512:(sub + 1) * 512], ps[:])
                            nc.vector.tensor_reduce(blockmax[:, kh:kh + 1], blk[:],
                                                    axis=mybir.AxisListType.X,
                                                    op=ALU.max)
                            blocks.append(blk)
                        rowmax = stpool.tile([128, 1], F32, tag="rm")
                        nc.vector.tensor_reduce(rowmax[:], blockmax[:],
                                                axis=mybir.AxisListType.X, op=ALU.max)
                        negmax = stpool.tile([128, 1], F32, tag="nm")
                        nc.vector.tensor_scalar_mul(negmax[:], rowmax[:], -1.0)
                        bfb = []
                        for kh in range(2):
                            if rnd == 0:
                                bb = scbpool.tile([128, 1024], BF16, tag="blkb",
                                                 name=f"bb{qi}_{kh}")
                                nc.scalar.activation(bb[:], blocks[kh][:], AF.Exp,
                                                     bias=negmax[:],
                                                     accum_out=sums[:, kh:kh + 1])
                                bfb.append(bb)
                            else:
                                nc.scalar.activation(blocks[kh][:], blocks[kh][:],
                                                     AF.Exp, bias=negmax[:],
                                                     accum_out=sums[:, kh:kh + 1])
                        rowsum = stpool.tile([128, 1], F32, tag="rs")
                        nc.vector.tensor_reduce(rowsum[:], sums[:],
                                                axis=mybir.AxisListType.X, op=ALU.add)
                        if rnd == 0:
                            nc.vector.tensor_copy(saved[qi][:, 0:1], rowmax[:])
                            nc.vector.tensor_copy(saved[qi][:, 1:2], rowsum[:])
                            for kh in range(2):
                                nc.gpsimd.dma_start(
                                    scratch[q0:q0 + 128, kh * 1024:(kh + 1) * 1024],
                                    bfb[kh][:])
                        else:
                            m1 = saved[qi][:, 0:1]
                            s1 = saved[qi][:, 1:2]
                            # negm = -max(m1, rowmax)
                            negm = stpool.tile([128, 1], F32, tag="ngm")
                            nc.vector.tensor_scalar(negm[:], rowmax[:], m1, -1.0,
                                                    op0=ALU.max, op1=ALU.mult)
                            e1 = stpool.tile([128, 1], F32, tag="e1")
                            nc.scalar.activation(e1[:], m1, AF.Exp, bias=negm[:])
                            e2 = stpool.tile([128, 1], F32, tag="e2")
                            nc.scalar.activation(e2[:], rowmax[:], AF.Exp, bias=negm[:])
                            # z = s2*e2 + (s1*e1)
                            t1 = stpool.tile([128, 1], F32, tag="t1")
                            nc.vector.tensor_tensor(t1[:], s1, e1[:], op=ALU.mult)
                            z = stpool.tile([128, 1], F32, tag="z")
                            nc.vector.scalar_tensor_tensor(z[:], rowsum[:], e2[:], t1[:],
                                                           op0=ALU.mult, op1=ALU.add)
                            rz = stpool.tile([128, 1], F32, tag="rz")
                            nc.vector.reciprocal(rz[:], z[:])
                            r1 = stpool.tile([128, 1], F32, tag="r1")
                            nc.vector.tensor_tensor(r1[:], e1[:], rz[:], op=ALU.mult)
                            r2 = stpool.tile([128, 1], F32, tag="r2")
                            nc.vector.tensor_tensor(r2[:], e2[:], rz[:], op=ALU.mult)
                            for kh in range(2):
                                nc.vector.tensor_scalar_mul(blocks[kh][:], blocks[kh][:],
                                                            r2[:])
                                nc.gpsimd.dma_start(
                                    out_d[q0:q0 + 128,
                                          KH + kh * 1024:KH + (kh + 1) * 1024],
                                    blocks[kh][:])
                            # rescale round-1 half from scratch
                            for kh in range(2):
                                fbb = fbpool.tile([128, 1024], BF16, tag="fbb",
                                                  name=f"fbb{qi}_{kh}")
                                nc.gpsimd.dma_start(
                                    fbb[:],
                                    scratch[q0:q0 + 128, kh * 1024:(kh + 1) * 1024])
                                fbf = fbpool.tile([128, 1024], F32, tag="fbf",
                                                  name=f"fbf{qi}_{kh}")
                                nc.vector.tensor_scalar_mul(fbf[:], fbb[:], r1[:])
                                nc.gpsimd.dma_start(
                                    out_d[q0:q0 + 128, kh * 1024:(kh + 1) * 1024],
                                    fbf[:])

            fbctx.close()

    nc.compile()
    _cache["nc"] = nc
    return nc


def kernel(hidden, pre_emb, w1, b1, w2, b2):
    hidden = np.ascontiguousarray(np.asarray(hidden, dtype=np.float32))
    pre_emb = np.ascontiguousarray(np.asarray(pre_emb, dtype=np.float32))
    w1 = np.ascontiguousarray(np.asarray(w1, dtype=np.float32))
    b1 = np.ascontiguousarray(np.asarray(b1, dtype=np.float32))
    w2 = np.ascontiguousarray(np.asarray(w2, dtype=np.float32))
    b2 = np.ascontiguousarray(np.asarray(b2, dtype=np.float32))

    nc = _build()
    in_maps = []
    for c in range(N_CORES):
        b, qh = c // 2, c % 2
        in_maps.append({
            "hid_q": np.ascontiguousarray(hidden[b, qh * QP:(qh + 1) * QP, :]),
            "hid_kv": hidden[b],
            "pre_kv": pre_emb[b],
            "w1": w1, "w2": w2, "b1": b1, "b2": b2,
        })
    kw = {}
    if TRACE:
        kw = dict(trace=True, trace_cores=[0])
    res = run_bass_kernel_spmd(nc, in_maps, core_ids=list(range(N_CORES)), **kw)
    global LAST_EXEC_NS
    if res.exec_time_ns is not None:
        LAST_EXEC_NS = res.exec_time_ns
    out = np.empty((B, S, S), dtype=np.float32)
    for c in range(N_CORES):
        b, qh = c // 2, c % 2
        out[b, qh * QP:(qh + 1) * QP, :] = res.results[c]["out"]
    return out
